# revision 13
# baseline (speedup 1.0000x reference)
"""Sparse (DAG-masked) attention head on 8 Trainium2 NeuronCores.

Reference computation (per batch b of 64):
    K = X_b @ Wk + bk; Q = Y_b @ Wq + bq; V = X_b @ Wv + bv         [T=1024, H=512]
    S = Q @ K^T / sqrt(H); A = softmax(where(dag.T*S == 0, -inf, dag.T*S))
    O = A @ V   (fully-masked rows -> 0)

Strategy: data-parallel over batch (8 batches per core); weights + dag
replicated. All matmuls run in float16 (1 cycle/row on PE with Fast
Weight Load, vs no-FWL for fp32/f32r; fp16's 11-bit mantissa keeps rel
err ~6e-4 and every intermediate is < 1e3, far from fp16 overflow).
Activations, weights, mask, and the output all move as f16 (the host
upcasts O to f32 after the gather) -- the kernel is paced by PE matmul
streaming with SBUF traffic close behind, so every byte counts.

Key algebraic fusion: softmax over s is invariant to additive terms that
vary only in t, so
    S^T[s,t] = (X G Y^T)[s,t] + beta[s] + (t-only terms, cancel)
with G = Wk @ Wq^T and beta = X @ (Wk @ bq), both folded on the host.
This removes one of the three projections and both K/Q bias adds.

Scores are computed TRANSPOSED (ST[s,t] = Z @ Y^T with Z^T = G^T X^T) so
the softmax weights PT = dag * exp(ST/sqrt(H) + beta*scale) land directly
in the [s, t] layout needed as the stationary operand of the P @ V
matmul -- no on-chip transposes. Softmax skips max-subtraction (scores
are ~N(0,1); exp cannot overflow fp16's 65504 at <6 sigma logits).

Two further matmul fusions kill all softmax bookkeeping matmuls:
 - beta rides as a 257th output column of the V projection (host appends
   SCALE*(Wk@bq) as an extra column of Wv), landing per-partition exactly
   where the exp bias needs it.
 - the softmax denominator l[t] = sum_s PT rides as an extra ones-column
   of V in the U = PT^T @ V matmul, landing per-partition in PSUM next to
   the U values it normalizes. No ones-stationary matmuls, no
   row->column DMA scatters.

Host-side prep: X/Y are transposed to [D, T] per batch (the PE contracts
over the partition dim).
"""

import numpy as np

import concourse.bass as bass
import concourse.mybir as mybir
import concourse.tile as tile
from concourse import bacc
from concourse.bass_utils import run_bass_kernel_spmd

B, T, D, H = 64, 1024, 512, 512
NCORES = 8
BPC = B // NCORES          # batches per core
DC = D // 128              # d chunks (4)
TC = T // 128              # t/s tiles (8)
HH = H // 2                # h half (256)
SCALE = 1.0 / float(np.sqrt(H))

f32 = mybir.dt.float32
f16 = mybir.dt.float16
EXP = mybir.ActivationFunctionType.Exp
COPY = mybir.ActivationFunctionType.Copy

_CACHED_NC = None

MM_DT = f16                # matmul operand dtype


def _build(reps=1, mm_dt=None):
    # reps>1 wraps the whole pipeline in a hardware loop that re-runs it on
    # the same data -- used only by the timing harness (wall-clock deltas
    # cancel the axon RPC overhead).
    dt = MM_DT if mm_dt is None else mm_dt
    nc = bacc.Bacc("TRN2", target_bir_lowering=False, debug=False,
                   num_devices=NCORES)

    XTd = nc.dram_tensor("XT", [BPC, DC, 128, T], dt, kind="ExternalInput").ap()
    YTd = nc.dram_tensor("YT", [BPC, DC, 128, T], dt, kind="ExternalInput").ap()
    DAGd = nc.dram_tensor("dagr", [TC, 128, T], dt, kind="ExternalInput").ap()
    Gd = nc.dram_tensor("Gr", [DC, 128, D], dt, kind="ExternalInput").ap()
    Wvd = nc.dram_tensor("Wvr", [DC, 128, 2, HH + 1], dt,
                         kind="ExternalInput").ap()
    Bvd = nc.dram_tensor("bvb", [128, H], dt, kind="ExternalInput").ap()
    Od = nc.dram_tensor("O", [BPC, T, H], dt, kind="ExternalOutput").ap()

    with tile.TileContext(nc) as tc:
        with (
            tc.tile_pool(name="const", bufs=1) as const,
            tc.tile_pool(name="xpool", bufs=2) as xpool,
            tc.tile_pool(name="ypool", bufs=2) as ypool,
            tc.tile_pool(name="zpool", bufs=2) as zpool,
            tc.tile_pool(name="vpool", bufs=2) as vpool,
            tc.tile_pool(name="pipe", bufs=2) as pipe,
            tc.tile_pool(name="small", bufs=3) as small,
            tc.tile_pool(name="psum", bufs=1, space="PSUM") as psum,
        ):
            # ---- resident tensors ----
            # SP queue: gt + batch-0 activations (consumption order);
            # Pool queue: V-weights, bias, dag mask.
            gt = const.tile([128, DC, D], dt, tag="gt")
            wv = const.tile([128, DC, 2, HH + 1], dt, tag="wv")
            bvb = const.tile([128, H], dt, tag="bvb")
            dag = const.tile([128, TC, T], dt, tag="dag")
            if reps == 1:
                xt0 = xpool.tile([128, DC, T], dt, tag="xt")
                yt0 = ypool.tile([128, DC, T], dt, tag="yt")
                for c in range(DC):
                    nc.sync.dma_start(out=gt[:, c], in_=Gd[c])
                    nc.sync.dma_start(out=xt0[:, c], in_=XTd[0, c])
                for c in range(DC):
                    nc.sync.dma_start(out=yt0[:, c], in_=YTd[0, c])
            else:
                xt0 = yt0 = None
                for c in range(DC):
                    nc.sync.dma_start(out=gt[:, c], in_=Gd[c])
            for c in range(DC):
                nc.gpsimd.dma_start(out=wv[:, c], in_=Wvd[c])
            nc.gpsimd.dma_start(out=bvb[:], in_=Bvd[:])
            for i in range(TC):
                nc.gpsimd.dma_start(out=dag[:, i], in_=DAGd[i])

            def emit_batch(b):
                # ---- load activations (transposed: [d, t]) ----
                if b == 0 and xt0 is not None:
                    xt, yt = xt0, yt0
                else:
                    xt = xpool.tile([128, DC, T], dt, tag="xt")
                    yt = ypool.tile([128, DC, T], dt, tag="yt")
                    for c in range(DC):
                        nc.sync.dma_start(out=xt[:, c], in_=XTd[b, c])
                    for c in range(DC):
                        nc.sync.dma_start(out=yt[:, c], in_=YTd[b, c])

                # ---- ZT[d', s] = G^T X^T: lhsT = G[d, d'_tile], rhs = XT ----
                zt = zpool.tile([128, DC, T], dt, tag="zt")
                for j in range(DC):
                    for hf in range(2):
                        ps = psum.tile([128, 512], f32, tag="mm", bufs=3)
                        for c in range(DC):
                            nc.tensor.matmul(
                                ps[:],
                                gt[:, c, j * 128:(j + 1) * 128],
                                xt[:, c, hf * 512:(hf + 1) * 512],
                                start=(c == 0), stop=(c == DC - 1),
                            )
                        nc.scalar.activation(
                            zt[:, j, hf * 512:(hf + 1) * 512], ps[:],
                            COPY, bias=0.0, scale=1.0,
                        )

                # ---- V[s, h] (+ beta as col 256 of half 0) ----
                # lhsT = XT[d, s_tile], rhs = [Wv_half | SCALE*Wk@bq]
                v = vpool.tile([128, TC, 2, HH + 1], dt, tag="v")
                nc.vector.memset(v[:, :, :, HH:HH + 1], 1.0)
                beta = small.tile([128, TC], f32, tag="beta")
                for i in range(TC):
                    for h in range(2):
                        w = HH + 1 if h == 0 else HH
                        ps = psum.tile([128, 512], f32, tag="mm", bufs=3)
                        for c in range(DC):
                            nc.tensor.matmul(
                                ps[:, :w],
                                xt[:, c, i * 128:(i + 1) * 128],
                                wv[:, c, h, :w],
                                start=(c == 0), stop=(c == DC - 1),
                            )
                        nc.vector.tensor_add(
                            v[:, i, h, :HH], ps[:, :HH],
                            bvb[:, h * HH:(h + 1) * HH])
                        if h == 0:
                            nc.vector.tensor_scalar_mul(
                                beta[:, i:i + 1], ps[:, HH:HH + 1], 1.0)

                # ---- scores, both t-halves: PT[s,t] = dag*exp(ST*scale+beta)
                pts = []
                for th in range(2):
                    t0 = th * 512
                    pt = pipe.tile([128, TC, 512], dt, tag=f"pt{th}")
                    pts.append(pt)
                    for i in range(TC):
                        ps = psum.tile([128, 512], f32, tag="mm", bufs=3)
                        for j in range(DC):
                            nc.tensor.matmul(
                                ps[:],
                                zt[:, j, i * 128:(i + 1) * 128],
                                yt[:, j, t0:t0 + 512],
                                start=(j == 0), stop=(j == DC - 1),
                            )
                        tmp = small.tile([128, 512], dt, tag="exp")
                        nc.scalar.activation(tmp[:], ps[:], EXP,
                                             bias=beta[:, i:i + 1],
                                             scale=SCALE)
                        nc.gpsimd.tensor_mul(
                            pt[:, i], tmp[:], dag[:, i, t0:t0 + 512],
                        )

                # ---- U = PT^T @ [V | ones]; O = U[:, :H] / U[:, H] ----
                for th in range(2):
                    pt = pts[th]
                    for tq in range(4):
                        t_ = th * 4 + tq
                        osb = small.tile([128, 512], dt, tag="osb")
                        linv = small.tile([128, 1], f32, tag="linv")
                        for h in range(2):
                            w = HH + 1 if h == 0 else HH
                            up = psum.tile([128, 512], f32, tag=f"u{h}",
                                           bufs=2)
                            for i in range(TC):
                                nc.tensor.matmul(
                                    up[:, :w],
                                    pt[:, i, tq * 128:(tq + 1) * 128],
                                    v[:, i, h, :w],
                                    start=(i == 0), stop=(i == TC - 1),
                                )
                            if h == 0:
                                lc = small.tile([128, 1], f32, tag="lc")
                                nc.vector.tensor_scalar_max(
                                    lc[:], up[:, HH:HH + 1], 1e-30)
                                nc.vector.reciprocal(linv[:], lc[:])
                            nc.scalar.activation(
                                osb[:, h * HH:(h + 1) * HH], up[:, :HH],
                                COPY, bias=0.0, scale=linv[:])
                        nc.gpsimd.dma_start(
                            out=Od[b, t_ * 128:(t_ + 1) * 128], in_=osb[:])

            if reps == 1:
                for b in range(BPC):
                    emit_batch(b)
            else:
                with tc.For_i(0, reps, 1):
                    for b in range(BPC):
                        emit_batch(b)

    nc.compile()
    return nc


def _get_nc():
    global _CACHED_NC
    if _CACHED_NC is None:
        _CACHED_NC = _build()
    return _CACHED_NC


def _prep_core_inputs(X, Y, dag, Wk, bk, Wq, bq, Wv, bv, mm_dt=None):
    """Build the 8 per-core input maps (host-side shard + transpose +
    weight fusion G = Wk Wq^T, beta column = SCALE * Wk bq)."""
    import ml_dtypes
    dt = MM_DT if mm_dt is None else mm_dt
    mmnp = {mybir.dt.bfloat16: ml_dtypes.bfloat16,
            f16: np.float16}.get(dt, np.float32)
    X = np.ascontiguousarray(np.asarray(X, dtype=np.float32))
    Y = np.ascontiguousarray(np.asarray(Y, dtype=np.float32))
    dag = np.ascontiguousarray(np.asarray(dag, dtype=np.float32))
    dag_r = dag.reshape(TC, 128, T).astype(mmnp)
    Wk64 = np.asarray(Wk, np.float64)
    G = (Wk64 @ np.asarray(Wq, np.float64).T).astype(np.float32)
    g = (SCALE * (Wk64 @ np.asarray(bq, np.float64))).astype(np.float32)
    Wvr = np.zeros((DC, 128, 2, HH + 1), dtype=mmnp)
    Wvf = np.asarray(Wv, np.float32).reshape(DC, 128, H)
    Wvr[:, :, 0, :HH] = Wvf[:, :, :HH]
    Wvr[:, :, 1, :HH] = Wvf[:, :, HH:]
    Wvr[:, :, 0, HH] = g.reshape(DC, 128)
    shared = {
        "dagr": dag_r,
        "Gr": G.reshape(DC, 128, D).astype(mmnp),
        "Wvr": Wvr,
        "bvb": np.ascontiguousarray(
            np.broadcast_to(np.asarray(bv, np.float32), (128, H))).astype(mmnp),
    }
    in_maps = []
    for core in range(NCORES):
        sl = slice(core * BPC, (core + 1) * BPC)
        xt = np.ascontiguousarray(X[sl].transpose(0, 2, 1)).reshape(
            BPC, DC, 128, T).astype(mmnp, copy=False)
        yt = np.ascontiguousarray(Y[sl].transpose(0, 2, 1)).reshape(
            BPC, DC, 128, T).astype(mmnp, copy=False)
        in_maps.append({"XT": xt, "YT": yt, **shared})
    return in_maps


def kernel(X, Y, dag, Wk, bk, Wq, bq, Wv, bv):
    nc = _get_nc()
    in_maps = _prep_core_inputs(X, Y, dag, Wk, bk, Wq, bq, Wv, bv)
    last_err = None
    for _attempt in range(3):
        try:
            res = run_bass_kernel_spmd(nc, in_maps, list(range(NCORES)))
            break
        except Exception as e:  # transient NRT device errors -- retry
            last_err = e
    else:
        raise last_err
    return np.concatenate([res.results[i]["O"] for i in range(NCORES)],
                          axis=0).astype(np.float32)


# revision 14
# speedup vs baseline: 1.0426x; 1.0426x over previous
"""Sparse (DAG-masked) attention head on 8 Trainium2 NeuronCores.

Reference computation (per batch b of 64):
    K = X_b @ Wk + bk; Q = Y_b @ Wq + bq; V = X_b @ Wv + bv         [T=1024, H=512]
    S = Q @ K^T / sqrt(H); A = softmax(where(dag.T*S == 0, -inf, dag.T*S))
    O = A @ V   (fully-masked rows -> 0)

Strategy: data-parallel over batch (8 batches per core); weights + dag
replicated. All matmuls run in float16 (1 cycle/row on PE with Fast
Weight Load, vs no-FWL for fp32/f32r; fp16's 11-bit mantissa keeps rel
err ~6e-4 and every intermediate is < 1e3, far from fp16 overflow).
Activations, weights, mask, and the output all move as f16 (the host
upcasts O to f32 after the gather) -- the kernel is paced by PE matmul
streaming with SBUF traffic close behind, so every byte counts.

Key algebraic fusion: softmax over s is invariant to additive terms that
vary only in t, so
    S^T[s,t] = (X G Y^T)[s,t] + beta[s] + (t-only terms, cancel)
with G = Wk @ Wq^T and beta = X @ (Wk @ bq), both folded on the host.
This removes one of the three projections and both K/Q bias adds.

Scores are computed TRANSPOSED (ST[s,t] = Z @ Y^T with Z^T = G^T X^T) so
the softmax weights PT = dag * exp(ST/sqrt(H) + beta*scale) land directly
in the [s, t] layout needed as the stationary operand of the P @ V
matmul -- no on-chip transposes. Softmax skips max-subtraction (scores
are ~N(0,1); exp cannot overflow fp16's 65504 at <6 sigma logits).

Two further matmul fusions kill all softmax bookkeeping matmuls:
 - beta rides as a 257th output column of the V projection (host appends
   SCALE*(Wk@bq) as an extra column of Wv), landing per-partition exactly
   where the exp bias needs it.
 - the softmax denominator l[t] = sum_s PT rides as an extra ones-column
   of V in the U = PT^T @ V matmul, landing per-partition in PSUM next to
   the U values it normalizes. No ones-stationary matmuls, no
   row->column DMA scatters.

Host-side prep: X/Y are transposed to [D, T] per batch (the PE contracts
over the partition dim).
"""

import numpy as np

import concourse.bass as bass
import concourse.mybir as mybir
import concourse.tile as tile
from concourse import bacc
from concourse.bass_utils import run_bass_kernel_spmd

B, T, D, H = 64, 1024, 512, 512
NCORES = 8
BPC = B // NCORES          # batches per core
DC = D // 128              # d chunks (4)
TC = T // 128              # t/s tiles (8)
HH = H // 2                # h half (256)
SCALE = 1.0 / float(np.sqrt(H))

f32 = mybir.dt.float32
f16 = mybir.dt.float16
EXP = mybir.ActivationFunctionType.Exp
COPY = mybir.ActivationFunctionType.Copy

_CACHED_NC = None

MM_DT = f16                # matmul operand dtype


def _build(reps=1, mm_dt=None):
    # reps>1 wraps the whole pipeline in a hardware loop that re-runs it on
    # the same data -- used only by the timing harness (wall-clock deltas
    # cancel the axon RPC overhead).
    dt = MM_DT if mm_dt is None else mm_dt
    nc = bacc.Bacc("TRN2", target_bir_lowering=False, debug=False,
                   num_devices=NCORES)

    XTd = nc.dram_tensor("XT", [BPC, DC, 128, T], dt, kind="ExternalInput").ap()
    YTd = nc.dram_tensor("YT", [BPC, DC, 128, T], dt, kind="ExternalInput").ap()
    DAGd = nc.dram_tensor("dagr", [TC, 128, T], dt, kind="ExternalInput").ap()
    Gd = nc.dram_tensor("Gr", [DC, 128, D], dt, kind="ExternalInput").ap()
    Wvd = nc.dram_tensor("Wvr", [DC, 128, 2, HH + 1], dt,
                         kind="ExternalInput").ap()
    Bvd = nc.dram_tensor("bvb", [128, H], dt, kind="ExternalInput").ap()
    Od = nc.dram_tensor("O", [BPC, T, H], dt, kind="ExternalOutput").ap()

    with tile.TileContext(nc) as tc:
        with (
            tc.tile_pool(name="const", bufs=1) as const,
            tc.tile_pool(name="xpool", bufs=3) as xpool,
            tc.tile_pool(name="ypool", bufs=3) as ypool,
            tc.tile_pool(name="zpool", bufs=2) as zpool,
            tc.tile_pool(name="vpool", bufs=3) as vpool,
            tc.tile_pool(name="pipe", bufs=2) as pipe,
            tc.tile_pool(name="small", bufs=4) as small,
            tc.tile_pool(name="psum", bufs=1, space="PSUM") as psum,
        ):
            # ---- resident tensors ----
            # SP queue: gt + batch-0 activations (consumption order);
            # Pool queue: V-weights, bias, dag mask.
            gt = const.tile([128, DC, D], dt, tag="gt")
            wv = const.tile([128, DC, 2, HH + 1], dt, tag="wv")
            bvb = const.tile([128, H], dt, tag="bvb")
            dag = const.tile([128, TC, T], dt, tag="dag")
            if reps == 1:
                xt0 = xpool.tile([128, DC, T], dt, tag="xt")
                yt0 = ypool.tile([128, DC, T], dt, tag="yt")
                for c in range(DC):
                    nc.sync.dma_start(out=gt[:, c], in_=Gd[c])
                    nc.sync.dma_start(out=xt0[:, c], in_=XTd[0, c])
                for c in range(DC):
                    nc.sync.dma_start(out=yt0[:, c], in_=YTd[0, c])
            else:
                xt0 = yt0 = None
                for c in range(DC):
                    nc.sync.dma_start(out=gt[:, c], in_=Gd[c])
            for c in range(DC):
                nc.gpsimd.dma_start(out=wv[:, c], in_=Wvd[c])
            nc.gpsimd.dma_start(out=bvb[:], in_=Bvd[:])
            for i in range(TC):
                nc.gpsimd.dma_start(out=dag[:, i], in_=DAGd[i])

            def emit_batch(b):
                # ---- load activations (transposed: [d, t]) ----
                if b == 0 and xt0 is not None:
                    xt, yt = xt0, yt0
                else:
                    xt = xpool.tile([128, DC, T], dt, tag="xt")
                    yt = ypool.tile([128, DC, T], dt, tag="yt")
                    for c in range(DC):
                        nc.sync.dma_start(out=xt[:, c], in_=XTd[b, c])
                    for c in range(DC):
                        nc.sync.dma_start(out=yt[:, c], in_=YTd[b, c])

                # ---- ZT[d', s] = G^T X^T: lhsT = G[d, d'_tile], rhs = XT ----
                zt = zpool.tile([128, DC, T], dt, tag="zt")
                for j in range(DC):
                    for hf in range(2):
                        ps = psum.tile([128, 512], f32, tag="mm", bufs=3)
                        for c in range(DC):
                            nc.tensor.matmul(
                                ps[:],
                                gt[:, c, j * 128:(j + 1) * 128],
                                xt[:, c, hf * 512:(hf + 1) * 512],
                                start=(c == 0), stop=(c == DC - 1),
                            )
                        nc.scalar.activation(
                            zt[:, j, hf * 512:(hf + 1) * 512], ps[:],
                            COPY, bias=0.0, scale=1.0,
                        )

                # ---- V[s, h] (+ beta as col 256 of half 0) ----
                # lhsT = XT[d, s_tile], rhs = [Wv_half | SCALE*Wk@bq]
                v = vpool.tile([128, TC, 2, HH + 1], dt, tag="v")
                nc.vector.memset(v[:, :, :, HH:HH + 1], 1.0)
                beta = small.tile([128, TC], f32, tag="beta")
                for i in range(TC):
                    for h in range(2):
                        w = HH + 1 if h == 0 else HH
                        ps = psum.tile([128, 512], f32, tag="mm", bufs=3)
                        for c in range(DC):
                            nc.tensor.matmul(
                                ps[:, :w],
                                xt[:, c, i * 128:(i + 1) * 128],
                                wv[:, c, h, :w],
                                start=(c == 0), stop=(c == DC - 1),
                            )
                        nc.vector.tensor_add(
                            v[:, i, h, :HH], ps[:, :HH],
                            bvb[:, h * HH:(h + 1) * HH])
                        if h == 0:
                            nc.vector.tensor_scalar_mul(
                                beta[:, i:i + 1], ps[:, HH:HH + 1], 1.0)

                # ---- scores, both t-halves: PT[s,t] = dag*exp(ST*scale+beta)
                pts = []
                for th in range(2):
                    t0 = th * 512
                    pt = pipe.tile([128, TC, 512], dt, tag=f"pt{th}")
                    pts.append(pt)
                    for i in range(TC):
                        ps = psum.tile([128, 512], f32, tag="mm", bufs=3)
                        for j in range(DC):
                            nc.tensor.matmul(
                                ps[:],
                                zt[:, j, i * 128:(i + 1) * 128],
                                yt[:, j, t0:t0 + 512],
                                start=(j == 0), stop=(j == DC - 1),
                            )
                        tmp = small.tile([128, 512], dt, tag="exp")
                        nc.scalar.activation(tmp[:], ps[:], EXP,
                                             bias=beta[:, i:i + 1],
                                             scale=SCALE)
                        nc.vector.tensor_mul(
                            pt[:, i], tmp[:], dag[:, i, t0:t0 + 512],
                        )

                # ---- U = PT^T @ [V | ones]; O = U[:, :H] / U[:, H] ----
                for th in range(2):
                    pt = pts[th]
                    for tq in range(4):
                        t_ = th * 4 + tq
                        osb = small.tile([128, 512], dt, tag="osb")
                        linv = small.tile([128, 1], f32, tag="linv")
                        for h in range(2):
                            w = HH + 1 if h == 0 else HH
                            up = psum.tile([128, 512], f32, tag=f"u{h}",
                                           bufs=2)
                            for i in range(TC):
                                nc.tensor.matmul(
                                    up[:, :w],
                                    pt[:, i, tq * 128:(tq + 1) * 128],
                                    v[:, i, h, :w],
                                    start=(i == 0), stop=(i == TC - 1),
                                )
                            if h == 0:
                                lc = small.tile([128, 1], f32, tag="lc")
                                nc.vector.tensor_scalar_max(
                                    lc[:], up[:, HH:HH + 1], 1e-30)
                                nc.vector.reciprocal(linv[:], lc[:])
                            nc.scalar.activation(
                                osb[:, h * HH:(h + 1) * HH], up[:, :HH],
                                COPY, bias=0.0, scale=linv[:])
                        nc.gpsimd.dma_start(
                            out=Od[b, t_ * 128:(t_ + 1) * 128], in_=osb[:])

            if reps == 1:
                for b in range(BPC):
                    emit_batch(b)
            else:
                with tc.For_i(0, reps, 1):
                    for b in range(BPC):
                        emit_batch(b)

    nc.compile()
    return nc


def _get_nc():
    global _CACHED_NC
    if _CACHED_NC is None:
        _CACHED_NC = _build()
    return _CACHED_NC


def _prep_core_inputs(X, Y, dag, Wk, bk, Wq, bq, Wv, bv, mm_dt=None):
    """Build the 8 per-core input maps (host-side shard + transpose +
    weight fusion G = Wk Wq^T, beta column = SCALE * Wk bq)."""
    import ml_dtypes
    dt = MM_DT if mm_dt is None else mm_dt
    mmnp = {mybir.dt.bfloat16: ml_dtypes.bfloat16,
            f16: np.float16}.get(dt, np.float32)
    X = np.ascontiguousarray(np.asarray(X, dtype=np.float32))
    Y = np.ascontiguousarray(np.asarray(Y, dtype=np.float32))
    dag = np.ascontiguousarray(np.asarray(dag, dtype=np.float32))
    dag_r = dag.reshape(TC, 128, T).astype(mmnp)
    Wk64 = np.asarray(Wk, np.float64)
    G = (Wk64 @ np.asarray(Wq, np.float64).T).astype(np.float32)
    g = (SCALE * (Wk64 @ np.asarray(bq, np.float64))).astype(np.float32)
    Wvr = np.zeros((DC, 128, 2, HH + 1), dtype=mmnp)
    Wvf = np.asarray(Wv, np.float32).reshape(DC, 128, H)
    Wvr[:, :, 0, :HH] = Wvf[:, :, :HH]
    Wvr[:, :, 1, :HH] = Wvf[:, :, HH:]
    Wvr[:, :, 0, HH] = g.reshape(DC, 128)
    shared = {
        "dagr": dag_r,
        "Gr": G.reshape(DC, 128, D).astype(mmnp),
        "Wvr": Wvr,
        "bvb": np.ascontiguousarray(
            np.broadcast_to(np.asarray(bv, np.float32), (128, H))).astype(mmnp),
    }
    in_maps = []
    for core in range(NCORES):
        sl = slice(core * BPC, (core + 1) * BPC)
        xt = np.ascontiguousarray(X[sl].transpose(0, 2, 1)).reshape(
            BPC, DC, 128, T).astype(mmnp, copy=False)
        yt = np.ascontiguousarray(Y[sl].transpose(0, 2, 1)).reshape(
            BPC, DC, 128, T).astype(mmnp, copy=False)
        in_maps.append({"XT": xt, "YT": yt, **shared})
    return in_maps


def kernel(X, Y, dag, Wk, bk, Wq, bq, Wv, bv):
    nc = _get_nc()
    in_maps = _prep_core_inputs(X, Y, dag, Wk, bk, Wq, bq, Wv, bv)
    last_err = None
    for _attempt in range(3):
        try:
            res = run_bass_kernel_spmd(nc, in_maps, list(range(NCORES)))
            break
        except Exception as e:  # transient NRT device errors -- retry
            last_err = e
    else:
        raise last_err
    return np.concatenate([res.results[i]["O"] for i in range(NCORES)],
                          axis=0).astype(np.float32)


# revision 15
# speedup vs baseline: 1.0514x; 1.0084x over previous
"""Sparse (DAG-masked) attention head on 8 Trainium2 NeuronCores.

Reference computation (per batch b of 64):
    K = X_b @ Wk + bk; Q = Y_b @ Wq + bq; V = X_b @ Wv + bv         [T=1024, H=512]
    S = Q @ K^T / sqrt(H); A = softmax(where(dag.T*S == 0, -inf, dag.T*S))
    O = A @ V   (fully-masked rows -> 0)

Strategy: data-parallel over batch (8 batches per core); weights + dag
replicated. All matmuls run in float16 (1 cycle/row on PE with Fast
Weight Load hiding LDWEIGHTS; fp16's 11-bit mantissa keeps rel err
~1e-3 and every intermediate is < 1e4, far from fp16 overflow).

Key algebraic fusion: softmax over s is invariant to additive terms that
vary only in t, so
    S^T[s,t] = (X G Y^T)[s,t] + beta[s] + (t-only terms, cancel)
with G = Wk @ Wq^T and beta = X @ (Wk @ bq), both folded on the host.
This removes one of the three projections and both K/Q bias adds.

Scores are computed TRANSPOSED (ST[s,t] = Z @ Y^T with Z^T = G^T X^T) so
the softmax weights PT = dag * exp(ST/sqrt(H) + beta*scale) land directly
in the [s, t] layout needed as the stationary operand of the P @ V
matmul -- no on-chip transposes. Softmax skips max-subtraction (scores
are ~N(0,1); exp cannot overflow fp16's 65504 at <6 sigma logits).

Two further matmul fusions kill all softmax bookkeeping matmuls:
 - beta rides as a 257th output column of the V projection (host appends
   SCALE*(Wk@bq) as an extra column of Wv), landing per-partition exactly
   where the exp bias needs it.
 - the softmax denominator l[t] = sum_s PT rides as an extra ones-column
   of V in the U = PT^T @ V matmul, landing per-partition in PSUM next to
   the U values it normalizes. No ones-stationary matmuls, no
   row->column DMA scatters.

Host-side prep: X/Y are transposed to [D, T] per batch (the PE contracts
over the partition dim).
"""

import numpy as np

import concourse.bass as bass
import concourse.mybir as mybir
import concourse.tile as tile
from concourse import bacc
from concourse.bass_utils import run_bass_kernel_spmd

B, T, D, H = 64, 1024, 512, 512
NCORES = 8
BPC = B // NCORES          # batches per core
DC = D // 128              # d chunks (4)
TC = T // 128              # t/s tiles (8)
HH = H // 2                # h half (256)
SCALE = 1.0 / float(np.sqrt(H))

f32 = mybir.dt.float32
f16 = mybir.dt.float16
EXP = mybir.ActivationFunctionType.Exp
COPY = mybir.ActivationFunctionType.Copy

_CACHED_NC = None

MM_DT = f16                # matmul operand dtype


def _build(reps=1, mm_dt=None):
    # reps>1 wraps the whole pipeline in a hardware loop that re-runs it on
    # the same data -- used only by the timing harness (wall-clock deltas
    # cancel the axon RPC overhead).
    dt = MM_DT if mm_dt is None else mm_dt
    nc = bacc.Bacc("TRN2", target_bir_lowering=False, debug=False,
                   num_devices=NCORES)

    XTd = nc.dram_tensor("XT", [BPC, DC, 128, T], dt, kind="ExternalInput").ap()
    YTd = nc.dram_tensor("YT", [BPC, DC, 128, T], dt, kind="ExternalInput").ap()
    DAGd = nc.dram_tensor("dagr", [TC, 128, T], dt, kind="ExternalInput").ap()
    Gd = nc.dram_tensor("Gr", [DC, 128, D], dt, kind="ExternalInput").ap()
    Wvd = nc.dram_tensor("Wvr", [DC, 128, 2, HH + 1], dt,
                         kind="ExternalInput").ap()
    Bvd = nc.dram_tensor("bvb", [128, H], f32, kind="ExternalInput").ap()
    Od = nc.dram_tensor("O", [BPC, T, H], f32, kind="ExternalOutput").ap()

    with tile.TileContext(nc) as tc:
        with (
            tc.tile_pool(name="const", bufs=1) as const,
            tc.tile_pool(name="xpool", bufs=2) as xpool,
            tc.tile_pool(name="ypool", bufs=2) as ypool,
            tc.tile_pool(name="zpool", bufs=2) as zpool,
            tc.tile_pool(name="vpool", bufs=2) as vpool,
            tc.tile_pool(name="pipe", bufs=2) as pipe,
            tc.tile_pool(name="small", bufs=3) as small,
            tc.tile_pool(name="psum", bufs=1, space="PSUM") as psum,
        ):
            # ---- resident tensors ----
            # SP queue: gt + batch-0 activations (consumption order);
            # Pool queue: V-weights, bias, dag mask.
            gt = const.tile([128, DC, D], dt, tag="gt")
            wv = const.tile([128, DC, 2, HH + 1], dt, tag="wv")
            bvb = const.tile([128, H], f32, tag="bvb")
            dag = const.tile([128, TC, T], dt, tag="dag")
            if reps == 1:
                xt0 = xpool.tile([128, DC, T], dt, tag="xt")
                yt0 = ypool.tile([128, DC, T], dt, tag="yt")
                for c in range(DC):
                    nc.sync.dma_start(out=gt[:, c], in_=Gd[c])
                    nc.sync.dma_start(out=xt0[:, c], in_=XTd[0, c])
                for c in range(DC):
                    nc.sync.dma_start(out=yt0[:, c], in_=YTd[0, c])
            else:
                xt0 = yt0 = None
                for c in range(DC):
                    nc.sync.dma_start(out=gt[:, c], in_=Gd[c])
            for c in range(DC):
                nc.gpsimd.dma_start(out=wv[:, c], in_=Wvd[c])
            nc.gpsimd.dma_start(out=bvb[:], in_=Bvd[:])
            for i in range(TC):
                nc.gpsimd.dma_start(out=dag[:, i], in_=DAGd[i])

            def emit_batch(b):
                # ---- load activations (transposed: [d, t]) ----
                if b == 0 and xt0 is not None:
                    xt, yt = xt0, yt0
                else:
                    xt = xpool.tile([128, DC, T], dt, tag="xt")
                    yt = ypool.tile([128, DC, T], dt, tag="yt")
                    for c in range(DC):
                        nc.sync.dma_start(out=xt[:, c], in_=XTd[b, c])
                    for c in range(DC):
                        nc.sync.dma_start(out=yt[:, c], in_=YTd[b, c])

                # ---- ZT[d', s] = G^T X^T: lhsT = G[d, d'_tile], rhs = XT ----
                zt = zpool.tile([128, DC, T], dt, tag="zt")
                for j in range(DC):
                    for hf in range(2):
                        ps = psum.tile([128, 512], f32, tag="mm", bufs=3)
                        for c in range(DC):
                            nc.tensor.matmul(
                                ps[:],
                                gt[:, c, j * 128:(j + 1) * 128],
                                xt[:, c, hf * 512:(hf + 1) * 512],
                                start=(c == 0), stop=(c == DC - 1),
                            )
                        nc.scalar.activation(
                            zt[:, j, hf * 512:(hf + 1) * 512], ps[:],
                            COPY, bias=0.0, scale=1.0,
                        )

                # ---- V[s, h] (+ beta as col 256 of half 0) ----
                # lhsT = XT[d, s_tile], rhs = [Wv_half | SCALE*Wk@bq]
                v = vpool.tile([128, TC, 2, HH + 1], dt, tag="v")
                nc.vector.memset(v[:, :, :, HH:HH + 1], 1.0)
                beta = small.tile([128, TC], f32, tag="beta")
                for i in range(TC):
                    for h in range(2):
                        w = HH + 1 if h == 0 else HH
                        ps = psum.tile([128, 512], f32, tag="mm", bufs=3)
                        for c in range(DC):
                            nc.tensor.matmul(
                                ps[:, :w],
                                xt[:, c, i * 128:(i + 1) * 128],
                                wv[:, c, h, :w],
                                start=(c == 0), stop=(c == DC - 1),
                            )
                        nc.vector.tensor_add(
                            v[:, i, h, :HH], ps[:, :HH],
                            bvb[:, h * HH:(h + 1) * HH])
                        if h == 0:
                            nc.vector.tensor_scalar_mul(
                                beta[:, i:i + 1], ps[:, HH:HH + 1], 1.0)

                # ---- scores, both t-halves: PT[s,t] = dag*exp(ST*scale+beta)
                pts = []
                for th in range(2):
                    t0 = th * 512
                    pt = pipe.tile([128, TC, 512], dt, tag=f"pt{th}")
                    pts.append(pt)
                    for i in range(TC):
                        ps = psum.tile([128, 512], f32, tag="mm", bufs=3)
                        for j in range(DC):
                            nc.tensor.matmul(
                                ps[:],
                                zt[:, j, i * 128:(i + 1) * 128],
                                yt[:, j, t0:t0 + 512],
                                start=(j == 0), stop=(j == DC - 1),
                            )
                        tmp = small.tile([128, 512], dt, tag="exp")
                        nc.scalar.activation(tmp[:], ps[:], EXP,
                                             bias=beta[:, i:i + 1],
                                             scale=SCALE)
                        nc.vector.tensor_mul(
                            pt[:, i], tmp[:], dag[:, i, t0:t0 + 512],
                        )

                # ---- U = PT^T @ [V | ones]; O = U[:, :H] / U[:, H] ----
                for th in range(2):
                    pt = pts[th]
                    for tq in range(4):
                        t_ = th * 4 + tq
                        osb = small.tile([128, 512], f32, tag="osb")
                        linv = small.tile([128, 1], f32, tag="linv")
                        for h in range(2):
                            w = HH + 1 if h == 0 else HH
                            up = psum.tile([128, 512], f32, tag=f"u{h}",
                                           bufs=2)
                            for i in range(TC):
                                nc.tensor.matmul(
                                    up[:, :w],
                                    pt[:, i, tq * 128:(tq + 1) * 128],
                                    v[:, i, h, :w],
                                    start=(i == 0), stop=(i == TC - 1),
                                )
                            if h == 0:
                                lc = small.tile([128, 1], f32, tag="lc")
                                nc.vector.tensor_scalar_max(
                                    lc[:], up[:, HH:HH + 1], 1e-30)
                                nc.vector.reciprocal(linv[:], lc[:])
                            nc.scalar.activation(
                                osb[:, h * HH:(h + 1) * HH], up[:, :HH],
                                COPY, bias=0.0, scale=linv[:])
                        nc.gpsimd.dma_start(
                            out=Od[b, t_ * 128:(t_ + 1) * 128], in_=osb[:])

            if reps == 1:
                for b in range(BPC):
                    emit_batch(b)
            else:
                with tc.For_i(0, reps, 1):
                    for b in range(BPC):
                        emit_batch(b)

    nc.compile()
    return nc


def _get_nc():
    global _CACHED_NC
    if _CACHED_NC is None:
        _CACHED_NC = _build()
    return _CACHED_NC


def _prep_core_inputs(X, Y, dag, Wk, bk, Wq, bq, Wv, bv, mm_dt=None):
    """Build the 8 per-core input maps (host-side shard + transpose +
    weight fusion G = Wk Wq^T, beta column = SCALE * Wk bq)."""
    import ml_dtypes
    dt = MM_DT if mm_dt is None else mm_dt
    mmnp = {mybir.dt.bfloat16: ml_dtypes.bfloat16,
            f16: np.float16}.get(dt, np.float32)
    X = np.ascontiguousarray(np.asarray(X, dtype=np.float32))
    Y = np.ascontiguousarray(np.asarray(Y, dtype=np.float32))
    dag = np.ascontiguousarray(np.asarray(dag, dtype=np.float32))
    dag_r = dag.reshape(TC, 128, T).astype(mmnp)
    Wk64 = np.asarray(Wk, np.float64)
    G = (Wk64 @ np.asarray(Wq, np.float64).T).astype(np.float32)
    g = (SCALE * (Wk64 @ np.asarray(bq, np.float64))).astype(np.float32)
    Wvr = np.zeros((DC, 128, 2, HH + 1), dtype=mmnp)
    Wvf = np.asarray(Wv, np.float32).reshape(DC, 128, H)
    Wvr[:, :, 0, :HH] = Wvf[:, :, :HH]
    Wvr[:, :, 1, :HH] = Wvf[:, :, HH:]
    Wvr[:, :, 0, HH] = g.reshape(DC, 128)
    shared = {
        "dagr": dag_r,
        "Gr": G.reshape(DC, 128, D).astype(mmnp),
        "Wvr": Wvr,
        "bvb": np.ascontiguousarray(
            np.broadcast_to(np.asarray(bv, np.float32), (128, H))),
    }
    in_maps = []
    for core in range(NCORES):
        sl = slice(core * BPC, (core + 1) * BPC)
        xt = np.ascontiguousarray(X[sl].transpose(0, 2, 1)).reshape(
            BPC, DC, 128, T).astype(mmnp, copy=False)
        yt = np.ascontiguousarray(Y[sl].transpose(0, 2, 1)).reshape(
            BPC, DC, 128, T).astype(mmnp, copy=False)
        in_maps.append({"XT": xt, "YT": yt, **shared})
    return in_maps


def kernel(X, Y, dag, Wk, bk, Wq, bq, Wv, bv):
    nc = _get_nc()
    in_maps = _prep_core_inputs(X, Y, dag, Wk, bk, Wq, bq, Wv, bv)
    last_err = None
    for _attempt in range(3):
        try:
            res = run_bass_kernel_spmd(nc, in_maps, list(range(NCORES)))
            break
        except Exception as e:  # transient NRT device errors -- retry
            last_err = e
    else:
        raise last_err
    return np.concatenate([res.results[i]["O"] for i in range(NCORES)],
                          axis=0)
